# revision 15
# baseline (speedup 1.0000x reference)
"""Trainium2 Bass kernel for the fused GNN message-passing block.

Reference computation (per batch b):
    h = silu(x @ W1 + b1) @ W2 + b2                       # [K, C]
    out[q, d, c] = sum_k mask[q,k] * ev[q,k,d] * ef[q,k,c] * h[k,c]

Sharding: data-parallel over (b, q-half) -> 8 cores, each core handles
one b (of 4) and 64 of the 128 q values.  All large tensors carry the
leading b dim; the tiny MLP weights are replicated.

Per-core device program (memory-bound; the ef slice is 8 MiB):
  - compute h via PE matmuls (x transposed on-chip with PE transposes,
    biases folded into the PSUM accumulation as rank-1 matmuls)
  - build w[k, d, q] = (mask * ev)^T via PE transposes + one DVE multiply
  - stream ef in [128(k), 8(q), 256(c)] tiles (1 MiB DMAs), multiply by
    h broadcast over q on DVE, then one tiny matmul per q on PE:
        out[d, c] = sum_k w[k, d, q] * (ef*h)[k, c]
    Four q-outputs are packed into one PSUM bank at partition offsets
    0/32/64/96 via tile_position col-groups so a single ACT copy drains
    four results at once.

The walrus build in this container accepts at most ONE sync wait per
instruction (setupSyncWait in CoreV3GenImpl), while Tile emits one wait
per dependent processor (the mandatory kernel-tail drain alone carries
~12).  _split_multiwaits() post-processes the finalized BIR: for every
instruction with N>1 waits it inserts N-1 single-wait NOPs immediately
before it on the same engine queue.  The sequencer executes waits in
queue order, so waiting serially on preceding NOPs is semantically
identical to the conjunctive multi-wait.
"""

import numpy as np

import concourse.bass as bass
import concourse.mybir as mybir
import concourse.tile as tile
from concourse.bass import ds, ts
from concourse.bass_utils import run_bass_kernel_spmd
from concourse.masks import make_identity

B, Q, K, D, C = 4, 128, 128, 3, 256
N_CORES = 8
QSH = Q // 2  # 64 q rows per core
QB = 8  # q values per ef tile (1 MiB DMA)
NG = QSH // QB
F32 = mybir.dt.float32

_NC_CACHE = {}


def _split_multiwaits(nc):
    """Legalize for the 1-sync-wait-per-instruction walrus: hoist all but
    the last wait of each instruction onto single-wait NOPs placed just
    before it on the same engine queue."""
    n = 0
    for f in nc.m.functions:
        for bb in f.blocks:
            out = []
            for inst in bb.instructions:
                si = inst.sync_info
                if si is not None and si.on_wait and len(si.on_wait) > 1:
                    waits = list(si.on_wait)
                    for w in waits[:-1]:
                        n += 1
                        nop = mybir.InstNoOp(
                            name=f"{inst.name}-wsplit{n}", ins=[], outs=[]
                        )
                        nop.engine = inst.engine
                        nop.sync_info = mybir.SyncInfo(on_wait=[w], on_update=[])
                        out.append(nop)
                    inst.sync_info = mybir.SyncInfo(
                        on_wait=[waits[-1]], on_update=list(si.on_update)
                    )
                out.append(inst)
            bb.instructions = out
    return nc


def _build_nc(split=True):
    nc = bass.Bass()

    ef_d = nc.declare_dram_parameter("ef", [QSH, K, C], F32, isOutput=False)
    evT_d = nc.declare_dram_parameter("evT", [K, D, QSH], F32, isOutput=False)
    maskT_d = nc.declare_dram_parameter("maskT", [K, QSH], F32, isOutput=False)
    xT_d = nc.declare_dram_parameter("xT", [C, K], F32, isOutput=False)
    w1_d = nc.declare_dram_parameter("W1", [C, C], F32, isOutput=False)
    b1_d = nc.declare_dram_parameter("b1", [C], F32, isOutput=False)
    w2_d = nc.declare_dram_parameter("W2", [C, C], F32, isOutput=False)
    b2_d = nc.declare_dram_parameter("b2", [C], F32, isOutput=False)
    # padded: one [128, 512] staging tile per 8-q group is DMAd verbatim
    # (row 32s+d, col 256f+c holds out[g*8+f*4+s, d, c]); host strips padding
    out_d = nc.declare_dram_parameter("out", [NG, 128, 2 * C], F32, isOutput=True)

    with tile.TileContext(nc) as tc:
        with (
            tc.tile_pool(name="const", bufs=1) as cpool,
            tc.tile_pool(name="efp", bufs=8) as efpool,
            tc.tile_pool(name="outp", bufs=3) as outpool,
            tc.tile_pool(name="pprep", bufs=1, space="PSUM") as pprep,
            tc.tile_pool(name="pout", bufs=5, space="PSUM") as pout,
        ):
            # ---- PE warm-up: ~3.4us of dep-light matmuls flips HAM to 8/8
            # before the MLP chain and main loop need the PE ----
            w_warm = cpool.tile([128, 2 * C], F32)
            nc.vector.memset(w_warm[:], 0.0)
            warm_ps = pout.tile([128, 2 * C], F32, tag="opsum", name="warm_ps")
            for _ in range(6):
                nc.tensor.matmul(
                    warm_ps[:, :C], w_warm[:, :128], w_warm[:, :C], start=True, stop=True
                )

            # ---- constants: xT/W1 on the SP queue (they gate the MLP),
            # everything else via SWDGE so ef prefetch isn't queued behind ----
            xT_sb = cpool.tile([128, 2, K], F32)
            nc.sync.dma_start(xT_sb[:], xT_d[:, :].rearrange("(o p) k -> p o k", p=128))
            w1_sb = cpool.tile([128, 2, C], F32)
            nc.sync.dma_start(w1_sb[:], w1_d[:, :].rearrange("(o p) n -> p o n", p=128))
            b1_sb = cpool.tile([1, C], F32)
            nc.gpsimd.dma_start(b1_sb[:], b1_d[:][None])
            b2_sb = cpool.tile([1, C], F32)
            nc.gpsimd.dma_start(b2_sb[:], b2_d[:][None])
            w2_sb = cpool.tile([128, 2, C], F32)
            nc.gpsimd.dma_start(w2_sb[:], w2_d[:, :].rearrange("(o p) n -> p o n", p=128))
            evT_sb = cpool.tile([K, D, QSH], F32)
            nc.gpsimd.dma_start(evT_sb[:], evT_d[:, :, :])
            maskT_sb = cpool.tile([K, QSH], F32)
            nc.gpsimd.dma_start(maskT_sb[:], maskT_d[:, :])
            ones_sb = cpool.tile([1, 128], F32)
            nc.gpsimd.memset(ones_sb[:], 1.0)

            # ---- MLP, fully transpose-free:
            # h1T[d,k] = (W1 slices)^T-style matmuls, silu in [d,k] layout,
            # then h[k,c] directly: lhsT = h1sT (natural), rhs = W2 (natural)
            h1T_ps = [
                pprep.tile([128, 128], F32, tag=f"prep{i}", name=f"h1T{i}")
                for i in range(2)
            ]
            for dh in range(2):
                nc.tensor.matmul(
                    h1T_ps[dh][:], w1_sb[:, 0, ts(dh, 128)], xT_sb[:, 0, :],
                    start=True, stop=False,
                )
                nc.tensor.matmul(
                    h1T_ps[dh][:], w1_sb[:, 1, ts(dh, 128)], xT_sb[:, 1, :],
                    start=False, stop=False,
                )
                nc.tensor.matmul(
                    h1T_ps[dh][:], b1_sb[:, ts(dh, 128)], ones_sb[:],
                    start=False, stop=True,
                )
            sigT_sb = cpool.tile([128, 2, 128], F32)
            h1sT_sb = cpool.tile([128, 2, 128], F32)
            for dh in range(2):
                nc.scalar.activation(
                    sigT_sb[:, dh], h1T_ps[dh][:], mybir.ActivationFunctionType.Sigmoid
                )
                nc.vector.tensor_tensor(
                    h1sT_sb[:, dh], h1T_ps[dh][:], sigT_sb[:, dh], mybir.AluOpType.mult
                )
            h_ps = pprep.tile([128, C], F32, tag="hps", name="h_ps")
            nc.tensor.matmul(h_ps[:], h1sT_sb[:, 0], w2_sb[:, 0], start=True, stop=False)
            nc.tensor.matmul(h_ps[:], h1sT_sb[:, 1], w2_sb[:, 1], start=False, stop=False)
            nc.tensor.matmul(h_ps[:], ones_sb[:], b2_sb[:], start=False, stop=True)
            h_sb = cpool.tile([128, C], F32)
            nc.scalar.copy(out=h_sb[:], in_=h_ps[:])

            # ---- w[k, q, 32] = (mask * ev)^T padded to 32 stationary columns
            # (cols d=3..31 stay zero so each matmul fills its whole 32-row
            # PSUM col-group and the drain copy never reads uninit PSUM) ----
            w_sb = cpool.tile([128, QSH, 32], F32)
            nc.gpsimd.memset(w_sb[:], 0.0)
            for d in range(D):
                nc.vector.tensor_copy(w_sb[:, :, d], evT_sb[:, d, :])
            nc.vector.tensor_tensor(
                w_sb[:, :, :D],
                w_sb[:, :, :D],
                maskT_sb[:, :, None].to_broadcast([K, QSH, D]),
                mybir.AluOpType.mult,
            )

            # ---- main loop over q groups ----
            for g in range(NG):
                ef_t = efpool.tile([K, QB, C], F32, tag="ef", name="ef_t")
                eng = nc.sync if g % 2 == 0 else nc.scalar
                eng.dma_start(ef_t[:], ef_d[ts(g, QB)].rearrange("q k c -> k q c"))
                # two groups run the big multiply on GpSimd (~2x slower than
                # DVE but otherwise idle) so the DVE chain isn't the long pole
                mul_eng = nc.gpsimd if g in (3, 6) else nc.vector
                mul_eng.tensor_tensor(
                    ef_t[:],
                    ef_t[:],
                    h_sb[:, None, :].to_broadcast([K, QB, C]),
                    mybir.AluOpType.mult,
                )
                ps = pout.tile([128, 2 * C], F32, tag="opsum", name="ps")
                for j in range(QB):
                    f, s = j // 4, j % 4
                    q = g * QB + j
                    nc.tensor.matmul(
                        ps[ds(32 * s, 32), ds(C * f, C)],
                        w_sb[:, q, :],
                        ef_t[:, j, :],
                        start=True,
                        stop=True,
                        tile_position=(0, 32 * s),
                    )
                o_sb = outpool.tile([128, 2 * C], F32, tag="osb", name="o_sb")
                nc.scalar.copy(out=o_sb[:], in_=ps[:])
                nc.gpsimd.dma_start(out_d[g], o_sb[:, :])

    return _split_multiwaits(nc) if split else nc


def _get_nc():
    if "nc" not in _NC_CACHE:
        _NC_CACHE["nc"] = _build_nc()
    return _NC_CACHE["nc"]


def _in_maps(inputs):
    x = np.asarray(inputs["x"], dtype=np.float32)
    ev = np.asarray(inputs["ev"], dtype=np.float32)
    ef = np.asarray(inputs["ef"], dtype=np.float32)
    am = np.asarray(inputs["access_mask"], dtype=np.float32)
    W1 = np.ascontiguousarray(np.asarray(inputs["W1"], dtype=np.float32))
    b1 = np.ascontiguousarray(np.asarray(inputs["b1"], dtype=np.float32))
    W2 = np.ascontiguousarray(np.asarray(inputs["W2"], dtype=np.float32))
    b2 = np.ascontiguousarray(np.asarray(inputs["b2"], dtype=np.float32))

    maps = []
    for core in range(N_CORES):
        b, qh = core // 2, core % 2
        sl = slice(qh * QSH, (qh + 1) * QSH)
        maps.append(
            {
                "ef": np.ascontiguousarray(ef[b, sl]),
                "evT": np.ascontiguousarray(ev[b, sl].transpose(1, 2, 0)),
                "maskT": np.ascontiguousarray(am[b, sl].T),
                "xT": np.ascontiguousarray(x[b].T),
                "W1": W1,
                "b1": b1,
                "W2": W2,
                "b2": b2,
            }
        )
    return maps


def _unpack_core(pad):
    # pad [NG, 128, 512]: row 32s+d, col 256f+c  ->  q = g*8 + f*4 + s
    v = pad.reshape(NG, 4, 32, 2, C)[:, :, :D, :, :]  # [g, s, d, f, c]
    return v.transpose(0, 3, 1, 2, 4).reshape(QSH, D, C)  # [g, f, s, d, c]


def _gather(results):
    out = np.empty((B, Q, D, C), dtype=np.float32)
    for core in range(N_CORES):
        b, qh = core // 2, core % 2
        out[b, qh * QSH : (qh + 1) * QSH] = _unpack_core(results[core]["out"])
    return out


def _run(inputs, trace=False, **kwargs):
    nc = _get_nc()
    res = run_bass_kernel_spmd(
        nc, _in_maps(inputs), list(range(N_CORES)), trace=trace, **kwargs
    )
    return _gather(res.results), res


def kernel(**inputs) -> np.ndarray:
    out, _ = _run(inputs, trace=False)
    return out


# revision 17
# speedup vs baseline: 1.1256x; 1.1256x over previous
"""Trainium2 Bass kernel for the fused GNN message-passing block.

Reference computation (per batch b):
    h = silu(x @ W1 + b1) @ W2 + b2                       # [K, C]
    out[q, d, c] = sum_k mask[q,k] * ev[q,k,d] * ef[q,k,c] * h[k,c]

Sharding: data-parallel over (b, q-half) -> 8 cores, each core handles
one b (of 4) and 64 of the 128 q values.  All large tensors carry the
leading b dim; the tiny MLP weights are replicated.

Per-core device program (memory-bound; the ef slice is 8 MiB):
  - compute h via PE matmuls (x transposed on-chip with PE transposes,
    biases folded into the PSUM accumulation as rank-1 matmuls)
  - build w[k, d, q] = (mask * ev)^T via PE transposes + one DVE multiply
  - stream ef in [128(k), 8(q), 256(c)] tiles (1 MiB DMAs), multiply by
    h broadcast over q on DVE, then one tiny matmul per q on PE:
        out[d, c] = sum_k w[k, d, q] * (ef*h)[k, c]
    Four q-outputs are packed into one PSUM bank at partition offsets
    0/32/64/96 via tile_position col-groups so a single ACT copy drains
    four results at once.

The walrus build in this container accepts at most ONE sync wait per
instruction (setupSyncWait in CoreV3GenImpl), while Tile emits one wait
per dependent processor (the mandatory kernel-tail drain alone carries
~12).  _split_multiwaits() post-processes the finalized BIR: for every
instruction with N>1 waits it inserts N-1 single-wait NOPs immediately
before it on the same engine queue.  The sequencer executes waits in
queue order, so waiting serially on preceding NOPs is semantically
identical to the conjunctive multi-wait.
"""

import numpy as np

import concourse.bass as bass
import concourse.mybir as mybir
import concourse.tile as tile
from concourse.bass import ds, ts
from concourse.bass_utils import run_bass_kernel_spmd
from concourse.masks import make_identity

B, Q, K, D, C = 4, 128, 128, 3, 256
N_CORES = 8
QSH = Q // 2  # 64 q rows per core
QB = 8  # q values per ef tile (1 MiB DMA)
NG = QSH // QB
F32 = mybir.dt.float32

_NC_CACHE = {}


def _split_multiwaits(nc):
    """Legalize for the 1-sync-wait-per-instruction walrus: hoist all but
    the last wait of each instruction onto single-wait NOPs placed just
    before it on the same engine queue."""
    n = 0
    for f in nc.m.functions:
        for bb in f.blocks:
            out = []
            for inst in bb.instructions:
                si = inst.sync_info
                if si is not None and si.on_wait and len(si.on_wait) > 1:
                    waits = list(si.on_wait)
                    for w in waits[:-1]:
                        n += 1
                        nop = mybir.InstNoOp(
                            name=f"{inst.name}-wsplit{n}", ins=[], outs=[]
                        )
                        nop.engine = inst.engine
                        nop.sync_info = mybir.SyncInfo(on_wait=[w], on_update=[])
                        out.append(nop)
                    inst.sync_info = mybir.SyncInfo(
                        on_wait=[waits[-1]], on_update=list(si.on_update)
                    )
                out.append(inst)
            bb.instructions = out
    return nc


def _build_nc(split=True):
    nc = bass.Bass()

    ef_d = nc.declare_dram_parameter("efT", [K, QSH, C], F32, isOutput=False)
    evT_d = nc.declare_dram_parameter("evT", [K, D, QSH], F32, isOutput=False)
    maskT_d = nc.declare_dram_parameter("maskT", [K, QSH], F32, isOutput=False)
    xT_d = nc.declare_dram_parameter("xT", [C, K], F32, isOutput=False)
    w1_d = nc.declare_dram_parameter("W1", [C, C], F32, isOutput=False)
    b1_d = nc.declare_dram_parameter("b1", [C], F32, isOutput=False)
    w2_d = nc.declare_dram_parameter("W2", [C, C], F32, isOutput=False)
    b2_d = nc.declare_dram_parameter("b2", [C], F32, isOutput=False)
    # padded: one [128, 512] staging tile per 8-q group is DMAd verbatim
    # (row 32s+d, col 256f+c holds out[g*8+f*4+s, d, c]); host strips padding
    out_d = nc.declare_dram_parameter("out", [NG, 128, 2 * C], F32, isOutput=True)

    with tile.TileContext(nc) as tc:
        with (
            tc.tile_pool(name="const", bufs=1) as cpool,
            tc.tile_pool(name="efp", bufs=1) as efpool,
            tc.tile_pool(name="outp", bufs=3) as outpool,
            tc.tile_pool(name="pprep", bufs=1, space="PSUM") as pprep,
            tc.tile_pool(name="pout", bufs=5, space="PSUM") as pout,
        ):
            # ---- PE warm-up: ~3.4us of dep-light matmuls flips HAM to 8/8
            # before the MLP chain and main loop need the PE ----
            w_warm = cpool.tile([128, 2 * C], F32)
            nc.vector.memset(w_warm[:], 0.0)
            warm_ps = pout.tile([128, 2 * C], F32, tag="opsum", name="warm_ps")
            for _ in range(6):
                nc.tensor.matmul(
                    warm_ps[:, :C], w_warm[:, :128], w_warm[:, :C], start=True, stop=True
                )

            # ---- constants: xT/W1 on the SP queue (they gate the MLP),
            # everything else via SWDGE so ef prefetch isn't queued behind ----
            xT_sb = cpool.tile([128, 2, K], F32)
            nc.sync.dma_start(xT_sb[:], xT_d[:, :].rearrange("(o p) k -> p o k", p=128))
            w1_sb = cpool.tile([128, 2, C], F32)
            nc.sync.dma_start(w1_sb[:], w1_d[:, :].rearrange("(o p) n -> p o n", p=128))
            b1_sb = cpool.tile([1, C], F32)
            nc.gpsimd.dma_start(b1_sb[:], b1_d[:][None])
            b2_sb = cpool.tile([1, C], F32)
            nc.gpsimd.dma_start(b2_sb[:], b2_d[:][None])
            w2_sb = cpool.tile([128, 2, C], F32)
            nc.gpsimd.dma_start(w2_sb[:], w2_d[:, :].rearrange("(o p) n -> p o n", p=128))
            evT_sb = cpool.tile([K, D, QSH], F32)
            nc.gpsimd.dma_start(evT_sb[:], evT_d[:, :, :])
            maskT_sb = cpool.tile([K, QSH], F32)
            nc.gpsimd.dma_start(maskT_sb[:], maskT_d[:, :])
            ones_sb = cpool.tile([1, 128], F32)
            nc.gpsimd.memset(ones_sb[:], 1.0)

            # ---- MLP, fully transpose-free:
            # h1T[d,k] = (W1 slices)^T-style matmuls, silu in [d,k] layout,
            # then h[k,c] directly: lhsT = h1sT (natural), rhs = W2 (natural)
            h1T_ps = [
                pprep.tile([128, 128], F32, tag=f"prep{i}", name=f"h1T{i}")
                for i in range(2)
            ]
            for dh in range(2):
                nc.tensor.matmul(
                    h1T_ps[dh][:], w1_sb[:, 0, ts(dh, 128)], xT_sb[:, 0, :],
                    start=True, stop=False,
                )
                nc.tensor.matmul(
                    h1T_ps[dh][:], w1_sb[:, 1, ts(dh, 128)], xT_sb[:, 1, :],
                    start=False, stop=False,
                )
                nc.tensor.matmul(
                    h1T_ps[dh][:], b1_sb[:, ts(dh, 128)], ones_sb[:],
                    start=False, stop=True,
                )
            sigT_sb = cpool.tile([128, 2, 128], F32)
            h1sT_sb = cpool.tile([128, 2, 128], F32)
            for dh in range(2):
                nc.scalar.activation(
                    sigT_sb[:, dh], h1T_ps[dh][:], mybir.ActivationFunctionType.Sigmoid
                )
                nc.vector.tensor_tensor(
                    h1sT_sb[:, dh], h1T_ps[dh][:], sigT_sb[:, dh], mybir.AluOpType.mult
                )
            h_ps = pprep.tile([128, C], F32, tag="hps", name="h_ps")
            nc.tensor.matmul(h_ps[:], h1sT_sb[:, 0], w2_sb[:, 0], start=True, stop=False)
            nc.tensor.matmul(h_ps[:], h1sT_sb[:, 1], w2_sb[:, 1], start=False, stop=False)
            nc.tensor.matmul(h_ps[:], ones_sb[:], b2_sb[:], start=False, stop=True)
            h_sb = cpool.tile([128, C], F32)
            nc.scalar.copy(out=h_sb[:], in_=h_ps[:])

            # ---- w[k, q, 32] = (mask * ev)^T padded to 32 stationary columns
            # (cols d=3..31 stay zero so each matmul fills its whole 32-row
            # PSUM col-group and the drain copy never reads uninit PSUM) ----
            w_sb = cpool.tile([128, QSH, 32], F32)
            nc.gpsimd.memset(w_sb[:], 0.0)
            for d in range(D):
                nc.vector.tensor_copy(w_sb[:, :, d], evT_sb[:, d, :])
            nc.vector.tensor_tensor(
                w_sb[:, :, :D],
                w_sb[:, :, :D],
                maskT_sb[:, :, None].to_broadcast([K, QSH, D]),
                mybir.AluOpType.mult,
            )

            # ---- main loop over q groups; all ef prefetches issued up-front
            # on the SP queue (contiguous 8KB per partition) ----
            ef_slots = [
                efpool.tile([K, QB, C], F32, tag=f"ef{g}", name=f"ef{g}")
                for g in range(NG)
            ]
            for g in range(NG):
                nc.sync.dma_start(ef_slots[g][:], ef_d[:, ts(g, QB), :])
            for g in range(NG):
                ef_t = ef_slots[g]
                nc.vector.tensor_tensor(
                    ef_t[:],
                    ef_t[:],
                    h_sb[:, None, :].to_broadcast([K, QB, C]),
                    mybir.AluOpType.mult,
                )
                ps = pout.tile([128, 2 * C], F32, tag="opsum", name="ps")
                for j in range(QB):
                    f, s = j // 4, j % 4
                    q = g * QB + j
                    nc.tensor.matmul(
                        ps[ds(32 * s, 32), ds(C * f, C)],
                        w_sb[:, q, :],
                        ef_t[:, j, :],
                        start=True,
                        stop=True,
                        tile_position=(0, 32 * s),
                    )
                o_sb = outpool.tile([128, 2 * C], F32, tag="osb", name="o_sb")
                nc.scalar.copy(out=o_sb[:], in_=ps[:])
                nc.gpsimd.dma_start(out_d[g], o_sb[:, :])

    return _split_multiwaits(nc) if split else nc


def _get_nc():
    if "nc" not in _NC_CACHE:
        _NC_CACHE["nc"] = _build_nc()
    return _NC_CACHE["nc"]


def _in_maps(inputs):
    x = np.asarray(inputs["x"], dtype=np.float32)
    ev = np.asarray(inputs["ev"], dtype=np.float32)
    ef = np.asarray(inputs["ef"], dtype=np.float32)
    am = np.asarray(inputs["access_mask"], dtype=np.float32)
    W1 = np.ascontiguousarray(np.asarray(inputs["W1"], dtype=np.float32))
    b1 = np.ascontiguousarray(np.asarray(inputs["b1"], dtype=np.float32))
    W2 = np.ascontiguousarray(np.asarray(inputs["W2"], dtype=np.float32))
    b2 = np.ascontiguousarray(np.asarray(inputs["b2"], dtype=np.float32))

    maps = []
    for core in range(N_CORES):
        b, qh = core // 2, core % 2
        sl = slice(qh * QSH, (qh + 1) * QSH)
        maps.append(
            {
                "efT": np.ascontiguousarray(ef[b, sl].transpose(1, 0, 2)),
                "evT": np.ascontiguousarray(ev[b, sl].transpose(1, 2, 0)),
                "maskT": np.ascontiguousarray(am[b, sl].T),
                "xT": np.ascontiguousarray(x[b].T),
                "W1": W1,
                "b1": b1,
                "W2": W2,
                "b2": b2,
            }
        )
    return maps


def _unpack_core(pad):
    # pad [NG, 128, 512]: row 32s+d, col 256f+c  ->  q = g*8 + f*4 + s
    v = pad.reshape(NG, 4, 32, 2, C)[:, :, :D, :, :]  # [g, s, d, f, c]
    return v.transpose(0, 3, 1, 2, 4).reshape(QSH, D, C)  # [g, f, s, d, c]


def _gather(results):
    out = np.empty((B, Q, D, C), dtype=np.float32)
    for core in range(N_CORES):
        b, qh = core // 2, core % 2
        out[b, qh * QSH : (qh + 1) * QSH] = _unpack_core(results[core]["out"])
    return out


def _run(inputs, trace=False, **kwargs):
    nc = _get_nc()
    res = run_bass_kernel_spmd(
        nc, _in_maps(inputs), list(range(N_CORES)), trace=trace, **kwargs
    )
    return _gather(res.results), res


def kernel(**inputs) -> np.ndarray:
    out, _ = _run(inputs, trace=False)
    return out
